# revision 15
# baseline (speedup 1.0000x reference)
"""Longformer (chunked sliding-window) self-attention on 8 TRN2 NeuronCores.

Sharding: sequence-parallel. B=2, L=4096 -> 8 blocks of 1024 query tokens
(4 per batch element), one block per core, each with a 512-token K/V halo
(the previous block). No cross-core communication.

v2 design (vs the bf16 baseline):
  - QKV projections run as fp8e4 DoubleRow matmuls with hi/lo error
    compensation: x and W are host-split into hi + lo (both e4m3) and the
    product takes 3 DR matmuls (hi*hi, hi*lo, lo*hi), each covering two
    128-row contraction tiles -> 0.75x the bf16 PE cost at better-than-
    bf16 accuracy (the hi/lo products are exact, fp32-accumulated).
  - Scores stay bf16: per-element fp8 noise on q/k transfers ~1:1 to the
    output through the softmax, which would blow the 2e-2 gate.
  - AV is reoriented: out[128 q, 65] = pT_tile.T @ [v | 1], so all 128
    PSUM partitions carry output (half the moving columns of the old
    [65, 512] orientation). Column 64 accumulates the softmax
    denominator, so normalization is a per-partition reciprocal +
    tensor_scalar_mul into a natural-layout ctx slab.
  - ctx slabs are transposed back to ctx^T for the output projection by
    the DMA xbar (InstDmaTransposeAnt) - zero compute-engine cost.
  - Halo keys are masked via the exp bias port: chunk-0 k-tiles 0..3 use
    a per-core [128,1] bias column (-1e9 on block-0 cores), making their
    probs exactly 0 in numerator and denominator. No v zeroing, and the
    v bias folds into the output bias on the host (bo' = bo + Wo @ bv).
  - Attention PE work (scores+AV) is ~55% of the exp stream's ACT time,
    so projection/output matmuls are interleaved as filler at k-tile
    granularity, and each pair's AV passes are software-pipelined into
    the next pair's score/exp window so the ACT stream never starves.
"""

import numpy as np

B, L, D = 2, 4096, 1024
H, DH, W = 16, 64, 512
NCORES = 8
BLK = L // 4          # 1024 query tokens per core
NKV = BLK + W         # 1536 kv tokens (halo + own)
CHUNKS = BLK // W     # 2 chunks per core
KT = (2 * W) // 128   # 8 k-token tiles of 128 per chunk window

_CACHE = {}


def _build():
    import concourse.bacc as bacc
    import concourse.mybir as mybir
    import concourse.tile as tile

    f32 = mybir.dt.float32
    bf16 = mybir.dt.bfloat16
    fp8 = mybir.dt.float8e4
    AF = mybir.ActivationFunctionType
    DR = mybir.MatmulPerfMode.DoubleRow

    nc = bacc.Bacc("TRN2", target_bir_lowering=False, debug=False,
                   num_devices=NCORES)

    xhi = nc.dram_tensor("xhi", [D, NKV], fp8, kind="ExternalInput").ap()
    xlo = nc.dram_tensor("xlo", [D, NKV], fp8, kind="ExternalInput").ap()
    wqh = nc.dram_tensor("wqh", [D, D], fp8, kind="ExternalInput").ap()
    wql = nc.dram_tensor("wql", [D, D], fp8, kind="ExternalInput").ap()
    wkh = nc.dram_tensor("wkh", [D, D], fp8, kind="ExternalInput").ap()
    wkl = nc.dram_tensor("wkl", [D, D], fp8, kind="ExternalInput").ap()
    wvh = nc.dram_tensor("wvh", [D, D], fp8, kind="ExternalInput").ap()
    wvl = nc.dram_tensor("wvl", [D, D], fp8, kind="ExternalInput").ap()
    woT = nc.dram_tensor("woT", [D, D], bf16, kind="ExternalInput").ap()
    bqr = nc.dram_tensor("bqr", [128, 8], f32, kind="ExternalInput").ap()
    bkr = nc.dram_tensor("bkr", [128, 8], f32, kind="ExternalInput").ap()
    borep = nc.dram_tensor("borep", [128, D], bf16,
                           kind="ExternalInput").ap()
    vbias = nc.dram_tensor("vbias", [128, 1], f32, kind="ExternalInput").ap()
    out = nc.dram_tensor("out", [BLK, D], f32, kind="ExternalOutput").ap()

    xhi_r = xhi.rearrange("(ko p) t -> p ko t", p=128)   # [128, 8, 1536]
    xlo_r = xlo.rearrange("(ko p) t -> p ko t", p=128)
    w_r = {n: t.rearrange("(ko p) d -> p ko d", p=128)
           for n, t in (("wqh", wqh), ("wql", wql), ("wkh", wkh),
                        ("wkl", wkl), ("wvh", wvh), ("wvl", wvl),
                        ("wo", woT))}
    out_r = out.rearrange("(to p) d -> p to d", p=128)   # [128, 8, 1024]

    # per-exp PE budget: exp [128,1024] = 1038 ns = 2491 PE cycles;
    # scores per kt = 1024 cy, one pipelined AV pass = 1040 cy.
    EXP_CY = 2491
    SCORE_CY = 1024
    AV_CY = 1040
    WSCALE_INV = 1.0 / 64.0

    with tile.TileContext(nc) as tc:
        with (
            tc.tile_pool(name="const", bufs=1) as constp,
            tc.tile_pool(name="xw", bufs=1) as xwp,
            tc.tile_pool(name="w8", bufs=1) as w8p,
            tc.tile_pool(name="wo", bufs=1) as wop,
            tc.tile_pool(name="acts", bufs=1) as actp,
            tc.tile_pool(name="ptiles", bufs=12) as pp,
            tc.tile_pool(name="ctxn", bufs=4) as ctxp,
            tc.tile_pool(name="recs", bufs=6) as recp,
            tc.tile_pool(name="outs", bufs=2) as op,
            tc.tile_pool(name="psS", bufs=2, space="PSUM") as psS,
            tc.tile_pool(name="psV", bufs=3, space="PSUM") as psV,
            tc.tile_pool(name="psA", bufs=1, space="PSUM") as psA,
        ):
            # ---- input DMA: own-token data first (chunk 1 runs first
            # and touches no halo); few, large transfers (HWDGE passes
            # serialize at ~630 ns each) ----
            bq_sb = constp.tile([128, 8], f32)
            bk_sb = constp.tile([128, 8], f32)
            vb_sb = constp.tile([128, 1], f32)
            bo_sb = constp.tile([128, D], bf16)
            nc.scalar.dma_start(bq_sb[:], bqr[:])
            nc.scalar.dma_start(bk_sb[:], bkr[:])
            nc.scalar.dma_start(vb_sb[:], vbias[:])
            nc.scalar.dma_start(bo_sb[:], borep[:])

            xh_sb = xwp.tile([128, 8, NKV], fp8, tag="xh")
            xl_sb = xwp.tile([128, 8, NKV], fp8, tag="xl")
            w_sb = {n: w8p.tile([128, 8, D], fp8, tag=n, name=n)
                    for n in ("wqh", "wql", "wkh", "wkl", "wvh", "wvl")}
            def wload(n, c0, c1):
                nc.sync.dma_start(w_sb[n][:, :, c0:c1], w_r[n][:, :, c0:c1])

            wload("wqh", 0, 128)
            nc.sync.dma_start(xh_sb[:, 0:2, 512:1536], xhi_r[:, 0:2, 512:1536])
            nc.sync.dma_start(xh_sb[:, 2:4, 512:1536], xhi_r[:, 2:4, 512:1536])
            nc.sync.dma_start(xh_sb[:, 4:6, 512:1536], xhi_r[:, 4:6, 512:1536])
            nc.sync.dma_start(xh_sb[:, 6:8, 512:1536], xhi_r[:, 6:8, 512:1536])
            wload("wql", 0, 128)
            for ko2 in range(4):
                nc.sync.dma_start(xl_sb[:, 2 * ko2:2 * ko2 + 2, 512:1536],
                                  xlo_r[:, 2 * ko2:2 * ko2 + 2, 512:1536])
            wload("wkh", 0, 128)
            wload("wkl", 0, 128)
            wload("wqh", 128, 512)
            wload("wql", 128, 512)
            wload("wkh", 128, 1024)
            wload("wkl", 128, 1024)
            wload("wvh", 0, 1024)
            wload("wvl", 0, 1024)
            wload("wqh", 512, 1024)
            wload("wql", 512, 1024)
            nc.scalar.dma_start(xh_sb[:, :, 0:512], xhi_r[:, :, 0:512])
            nc.scalar.dma_start(xl_sb[:, :, 0:512], xlo_r[:, :, 0:512])

            # ---- persistent activations ----
            q_sb = actp.tile([128, 8, BLK], bf16, tag="q")    # q^T [d, tok]
            k_sb = actp.tile([128, 8, NKV], bf16, tag="k")    # k^T [d, tok]
            # v natural [tok, h, dh+1]; col 64 per head = ones column that
            # accumulates the softmax denominator during AV.
            v_sb = actp.tile([128, 12, H * (DH + 1)], bf16, tag="v")
            v_v = v_sb[:].rearrange("p t (h e) -> p t h e", e=DH + 1)
            nc.vector.memset(v_v[:, :, :, DH], 1.0)
            # ctx^T [d, tok] for the out-projection (filled by dma xbar)
            ctxT_sb = actp.tile([128, 8, BLK], bf16, tag="ctxT")

            # ---- projections (fp8 DoubleRow hi/lo) ----
            # psA has a single bank; always alternate with a psV bank so a
            # unit's matmuls overlap the previous unit's DVE drain (a 1-ring
            # serializes on the write-after-read and costs ~950 ns/unit).
            ps_rot = [psA, psV]
            rot_i = [0]

            def dr_group(ps, lhs_hi, lhs_lo, rhs_hi, rhs_lo, lsl, rsl):
                """12 DR matmuls: hi*hi (4 ko-pair steps) + hi*lo + lo*hi."""
                for term, (lh, rh) in enumerate(
                        ((lhs_hi, rhs_hi), (lhs_hi, rhs_lo), (lhs_lo, rhs_hi))):
                    for s in range(4):
                        ksl = slice(2 * s, 2 * s + 2)
                        nc.tensor.matmul(
                            ps[:], lh[:, ksl, lsl], rh[:, ksl, rsl],
                            start=(term == 0 and s == 0),
                            stop=(term == 2 and s == 3), perf_mode=DR)

            def proj_ps(borrow=True):
                pool = ps_rot[rot_i[0] % 2]
                rot_i[0] += 1
                tag = "ps" if pool is psA else "av"
                return pool.tile([128, 512], f32, name="bps", tag=tag)

            def proj_qk_m(wh, wl, dst, bias, xn, dn, m, borrow=False):
                """One [128 dout x 512 tok] tile of a q^T/k^T projection."""
                ps = proj_ps(borrow)
                dr_group(ps, w_sb[wh], w_sb[wl], xh_sb[:], xl_sb[:],
                         slice(m * 128, (m + 1) * 128),
                         slice(xn * 512, (xn + 1) * 512))
                nc.vector.tensor_scalar(
                    dst[:, m, dn * 512:dn * 512 + 512], ps[:], WSCALE_INV,
                    bias[:, m:m + 1], mybir.AluOpType.mult,
                    mybir.AluOpType.add)

            def proj_v_t(t, n, borrow=False):
                """One [128 tok x 512 feature] tile of the v projection."""
                ps = proj_ps(borrow)
                dr_group(ps, xh_sb[:], xl_sb[:], w_sb["wvh"], w_sb["wvl"],
                         slice(t * 128, (t + 1) * 128),
                         slice(n * 512, (n + 1) * 512))
                nc.vector.tensor_scalar_mul(
                    v_v[:, t, n * 8:(n + 1) * 8, :DH],
                    ps[:].rearrange("p (h e) -> p h e", e=DH), WSCALE_INV)

            # ---- filler: PE work interleaved into the attention stream ----
            filler = []
            state = {"deficit": 0, "idx": 0}

            def add_filler(cycles, fn):
                filler.append((cycles, fn))

            def drain(cycles):
                state["deficit"] += cycles
                while (state["idx"] < len(filler)
                       and state["deficit"] >= filler[state["idx"]][0]):
                    cyc, fn = filler[state["idx"]]
                    state["idx"] += 1
                    state["deficit"] -= cyc
                    fn()

            def drain_to(idx):
                while state["idx"] < min(idx, len(filler)):
                    state["deficit"] = 0
                    cyc, fn = filler[state["idx"]]
                    state["idx"] += 1
                    fn()

            # ---- attention ----
            ctx_slabs = {}

            def transpose(c, qt):
                nc.sync.dma_start_transpose(
                    ctxT_sb[:, :, (c * 4 + qt) * 128:(c * 4 + qt + 1) * 128],
                    ctx_slabs[(c, qt)][:])

            def av_pass(c, u, plist, g, qta, qtb):
                """One AV pass: heads 2u+g, query tiles (qta, qtb)."""
                h = 2 * u + g
                tiles = []
                for qt in (qta, qtb):
                    tiles.append(psV.tile([128, 512], f32, name="av", tag="av"))
                for kt in range(KT):
                    vsl = v_v[:, c * 4 + kt, h, :]
                    for qt, av in zip((qta, qtb), tiles):
                        nc.tensor.matmul(
                            av[:, 0:65],
                            plist[kt][:, g * 512 + qt * 128:
                                      g * 512 + (qt + 1) * 128],
                            vsl, start=(kt == 0), stop=(kt == KT - 1))
                for qt, av in zip((qta, qtb), tiles):
                    rec = recp.tile([128, 1], f32, tag="rec", name="rec")
                    nc.vector.reciprocal(rec[:], av[:, 64:65])
                    nc.vector.tensor_scalar_mul(
                        ctx_slabs[(c, qt)][:, h * 64:(h + 1) * 64],
                        av[:, 0:64], rec[:, 0:1])

            PASSES = ((0, 0, 1), (1, 0, 1), (0, 2, 3), (1, 2, 3))
            pending = [None]      # (c, u, plist) awaiting AV
            pair_no = [0]

            def flush_pending(kt=None):
                """Emit AV pass kt (or all remaining) of the pending pair;
                after the chunk's last pair, transposes chase the passes."""
                if pending[0] is None:
                    return
                c, u, plist, done = pending[0]
                rng = range(4) if kt is None else [kt]
                for i in rng:
                    if i < done:
                        continue
                    av_pass(c, u, plist, *PASSES[i])
                    pending[0] = (c, u, plist, i + 1)
                    if u == 7 and i == 1:
                        transpose(c, 0)
                        transpose(c, 1)
                    if u == 7 and i == 3:
                        transpose(c, 2)
                        transpose(c, 3)
                if pending[0][3] >= 4:
                    pending[0] = None

            def emit_pair(c, u, force_idx=None):
                if force_idx is not None:
                    drain_to(force_idx)
                plist = []
                for kt in range(KT):
                    ksl = slice(c * 512 + kt * 128, c * 512 + (kt + 1) * 128)
                    qsl = slice(c * 512, (c + 1) * 512)
                    sps = psS.tile([128, 1024], f32, name="sps")
                    nc.tensor.matmul(sps[:, 0:512], k_sb[0:64, u, ksl],
                                     q_sb[0:64, u, qsl], start=True, stop=True)
                    nc.tensor.matmul(sps[:, 512:1024], k_sb[64:128, u, ksl],
                                     q_sb[64:128, u, qsl], start=True,
                                     stop=True)
                    p_t = pp.tile([128, 1024], bf16, tag="p", name="p")
                    if c == 0 and kt < 4:
                        nc.scalar.activation(p_t[:], sps[:], AF.Exp,
                                             bias=vb_sb[:, 0:1], scale=0.125)
                    else:
                        nc.scalar.activation(p_t[:], sps[:], AF.Exp,
                                             scale=0.125)
                    plist.append(p_t)
                    # early pairs are DMA-gated; mid pairs drain hard (ACT
                    # has slack); late pairs keep filler for the ACT-paced
                    # endgame.
                    pn = pair_no[0]
                    budget = 1300 if pn < 1 else 1550
                    if kt < 4 and pending[0] is not None:
                        pc, pu = pending[0][0], pending[0][1]
                        if kt == 0 and (pc, pu) in av_force:
                            drain_to(av_force[(pc, pu)])
                        flush_pending(kt)
                        budget -= AV_CY
                    drain(max(budget, 0))
                flush_pending()   # no-op unless fewer than 4 slots were free
                pending[0] = (c, u, plist, 0)
                pair_no[0] += 1

            def out_proj(to, n, borrow=False):
                ps = proj_ps(borrow)
                for ko in range(8):
                    nc.tensor.matmul(
                        ps[:], ctxT_sb[:, ko, to * 128:(to + 1) * 128],
                        wo_sb[:, ko, n * 512:(n + 1) * 512],
                        start=(ko == 0), stop=(ko == 7))
                o_t = op.tile([128, 512], f32, tag="o", name="o")
                nc.vector.tensor_add(o_t[:], ps[:],
                                     bo_sb[:, n * 512:(n + 1) * 512])
                nc.sync.dma_start(out_r[:, to, n * 512:(n + 1) * 512], o_t[:])

            # ---- phase schedule ----
            # Chunk 1 first: it touches no halo data, so the DMA stream
            # delivers own-token x/w first and the halo trails. All
            # projections beyond the first three m-tiles ride as filler
            # inside the attention stream (the exp stream is the pacer).
            wo_sb = wop.tile([128, 8, D], bf16)
            nc.sync.dma_start(wo_sb[:], w_r["wo"][:])

            # pre-attention: just what (1,0)'s first score tiles need;
            # K-s2-m0 (k-tiles 4..7) rides at the head of the filler.
            proj_qk_m("wqh", "wql", q_sb, bq_sb, 2, 1, 0, borrow=True)
            proj_qk_m("wkh", "wkl", k_sb, bk_sb, 1, 1, 0, borrow=True)

            score_force = {}
            av_force = {}
            add_filler(3072, lambda: proj_qk_m(
                "wkh", "wkl", k_sb, bk_sb, 2, 2, 0))

            def add_qk1(u):
                add_filler(3072, lambda m=u: proj_qk_m(
                    "wqh", "wql", q_sb, bq_sb, 2, 1, m))
                add_filler(3072, lambda m=u: proj_qk_m(
                    "wkh", "wkl", k_sb, bk_sb, 1, 1, m))
                add_filler(3072, lambda m=u: proj_qk_m(
                    "wkh", "wkl", k_sb, bk_sb, 2, 2, m))
                score_force[(1, u)] = len(filler)

            def add_qk0(u):
                add_filler(3072, lambda m=u: proj_qk_m(
                    "wqh", "wql", q_sb, bq_sb, 1, 0, m))
                add_filler(3072, lambda m=u: proj_qk_m(
                    "wkh", "wkl", k_sb, bk_sb, 0, 0, m))
                score_force[(0, u)] = len(filler)

            add_qk1(1)
            for t in range(4, 12):
                add_filler(3072, lambda t=t: proj_v_t(t, 0))
            av_force[(1, 0)] = len(filler)
            add_qk1(2)
            add_qk1(3)
            add_qk1(4)
            for t in range(4, 12):
                add_filler(3072, lambda t=t: proj_v_t(t, 1))
            av_force[(1, 4)] = len(filler)
            for u in (5, 6, 7):
                add_qk1(u)
            for u in range(8):
                add_qk0(u)
            for t in range(4):
                add_filler(3072, lambda t=t: proj_v_t(t, 0))
            av_force[(0, 0)] = len(filler)
            for t in range(4):
                add_filler(3072, lambda t=t: proj_v_t(t, 1))
            av_force[(0, 4)] = len(filler)
            for to in range(4, 8):
                for n in range(2):
                    add_filler(4096, lambda to=to, n=n: out_proj(to, n))

            for qt in range(4):
                ctx_slabs[(1, qt)] = ctxp.tile([128, BLK], bf16, tag="slab",
                                               name="slab")
            for u in range(8):
                emit_pair(1, u, force_idx=score_force.get((1, u)))
            for qt in range(4):
                ctx_slabs[(0, qt)] = ctxp.tile([128, BLK], bf16, tag="slab",
                                               name="slab")
            for u in range(8):
                emit_pair(0, u, force_idx=score_force.get((0, u)))

            flush_pending()
            drain_to(len(filler))
            for to in range(4):
                for n in range(2):
                    out_proj(to, n, borrow=True)

    nc.compile()
    return nc


def _host_prep(x, Wq, bq, Wk, bk, Wv, bv, Wo, bo):
    import ml_dtypes

    e4 = ml_dtypes.float8_e4m3
    bf = ml_dtypes.bfloat16

    def split8(a):
        a = np.ascontiguousarray(a, dtype=np.float32)
        hi = a.astype(e4)
        lo = (a - hi.astype(np.float32)).astype(e4)
        return hi, lo

    x = np.ascontiguousarray(np.asarray(x, dtype=np.float32))
    Wq = np.asarray(Wq, np.float32)
    Wk = np.asarray(Wk, np.float32)
    Wv = np.asarray(Wv, np.float32)
    Wo = np.asarray(Wo, np.float32)
    bv = np.asarray(bv, np.float32)
    bo = np.asarray(bo, np.float32)

    wqh, wql = split8(Wq.T * 64.0)
    wkh, wkl = split8(Wk.T * 64.0)
    wvh, wvl = split8(Wv.T * 64.0)
    bo_eff = bo + Wo @ bv          # v-bias folded through the attention avg
    mats = {
        "wqh": wqh, "wql": wql, "wkh": wkh, "wkl": wkl,
        "wvh": wvh, "wvl": wvl,
        "woT": np.ascontiguousarray(Wo.T.astype(bf)),
        "bqr": np.ascontiguousarray(
            np.asarray(bq, np.float32).reshape(8, 128).T),
        "bkr": np.ascontiguousarray(
            np.asarray(bk, np.float32).reshape(8, 128).T),
        "borep": np.ascontiguousarray(
            np.tile(bo_eff[None, :], (128, 1)).astype(bf)),
    }

    in_maps = []
    for core in range(NCORES):
        b, j = core // 4, core % 4
        start = j * BLK
        xkv = np.zeros((NKV, D), np.float32)
        lo = start - W
        if lo < 0:
            xkv[W:] = x[b, start:start + BLK]
        else:
            xkv[:] = x[b, lo:start + BLK]
        xh, xl = split8(xkv.T)
        vb = np.zeros((128, 1), np.float32)
        if j == 0:
            vb[:] = -1e9           # chunk-0 halo k-tiles masked in the exp
        im = dict(mats)
        im["xhi"] = xh
        im["xlo"] = xl
        im["vbias"] = vb
        in_maps.append(im)
    return in_maps


def kernel(x, Wq, bq, Wk, bk, Wv, bv, Wo, bo):
    from concourse.bass_utils import run_bass_kernel_spmd

    if "nc" not in _CACHE:
        _CACHE["nc"] = _build()
    nc = _CACHE["nc"]

    in_maps = _host_prep(x, Wq, bq, Wk, bk, Wv, bv, Wo, bo)
    res = run_bass_kernel_spmd(nc, in_maps, list(range(NCORES)))

    out = np.empty((B, L, D), np.float32)
    for core in range(NCORES):
        b, j = core // 4, core % 4
        out[b, j * BLK:(j + 1) * BLK] = res.results[core]["out"]
    return out


# revision 16
# speedup vs baseline: 1.0044x; 1.0044x over previous
"""Longformer (chunked sliding-window) self-attention on 8 TRN2 NeuronCores.

Sharding: sequence-parallel. B=2, L=4096 -> 8 blocks of 1024 query tokens
(4 per batch element), one block per core, each with a 512-token K/V halo
(the previous block). No cross-core communication.

v2 design (vs the bf16 baseline):
  - QKV projections run as fp8e4 DoubleRow matmuls with hi/lo error
    compensation: x and W are host-split into hi + lo (both e4m3) and the
    product takes 3 DR matmuls (hi*hi, hi*lo, lo*hi), each covering two
    128-row contraction tiles -> 0.75x the bf16 PE cost at better-than-
    bf16 accuracy (the hi/lo products are exact, fp32-accumulated).
  - Scores stay bf16: per-element fp8 noise on q/k transfers ~1:1 to the
    output through the softmax, which would blow the 2e-2 gate.
  - AV is reoriented: out[128 q, 65] = pT_tile.T @ [v | 1], so all 128
    PSUM partitions carry output (half the moving columns of the old
    [65, 512] orientation). Column 64 accumulates the softmax
    denominator, so normalization is a per-partition reciprocal +
    tensor_scalar_mul into a natural-layout ctx slab.
  - ctx slabs are transposed back to ctx^T for the output projection by
    the DMA xbar (InstDmaTransposeAnt) - zero compute-engine cost.
  - Halo keys are masked via the exp bias port: chunk-0 k-tiles 0..3 use
    a per-core [128,1] bias column (-1e9 on block-0 cores), making their
    probs exactly 0 in numerator and denominator. No v zeroing, and the
    v bias folds into the output bias on the host (bo' = bo + Wo @ bv).
  - Attention PE work (scores+AV) is ~55% of the exp stream's ACT time,
    so projection/output matmuls are interleaved as filler at k-tile
    granularity, and each pair's AV passes are software-pipelined into
    the next pair's score/exp window so the ACT stream never starves.
"""

import numpy as np

B, L, D = 2, 4096, 1024
H, DH, W = 16, 64, 512
NCORES = 8
BLK = L // 4          # 1024 query tokens per core
NKV = BLK + W         # 1536 kv tokens (halo + own)
CHUNKS = BLK // W     # 2 chunks per core
KT = (2 * W) // 128   # 8 k-token tiles of 128 per chunk window

_CACHE = {}


def _build():
    import concourse.bacc as bacc
    import concourse.mybir as mybir
    import concourse.tile as tile

    f32 = mybir.dt.float32
    bf16 = mybir.dt.bfloat16
    fp8 = mybir.dt.float8e4
    AF = mybir.ActivationFunctionType
    DR = mybir.MatmulPerfMode.DoubleRow

    nc = bacc.Bacc("TRN2", target_bir_lowering=False, debug=False,
                   num_devices=NCORES)

    xhi = nc.dram_tensor("xhi", [D, NKV], fp8, kind="ExternalInput").ap()
    xlo = nc.dram_tensor("xlo", [D, NKV], fp8, kind="ExternalInput").ap()
    wqh = nc.dram_tensor("wqh", [D, D], fp8, kind="ExternalInput").ap()
    wql = nc.dram_tensor("wql", [D, D], fp8, kind="ExternalInput").ap()
    wkh = nc.dram_tensor("wkh", [D, D], fp8, kind="ExternalInput").ap()
    wkl = nc.dram_tensor("wkl", [D, D], fp8, kind="ExternalInput").ap()
    wvh = nc.dram_tensor("wvh", [D, D], fp8, kind="ExternalInput").ap()
    wvl = nc.dram_tensor("wvl", [D, D], fp8, kind="ExternalInput").ap()
    woT = nc.dram_tensor("woT", [D, D], bf16, kind="ExternalInput").ap()
    bqr = nc.dram_tensor("bqr", [128, 8], f32, kind="ExternalInput").ap()
    bkr = nc.dram_tensor("bkr", [128, 8], f32, kind="ExternalInput").ap()
    borep = nc.dram_tensor("borep", [128, D], bf16,
                           kind="ExternalInput").ap()
    vbias = nc.dram_tensor("vbias", [128, 1], f32, kind="ExternalInput").ap()
    out = nc.dram_tensor("out", [BLK, D], f32, kind="ExternalOutput").ap()

    xhi_r = xhi.rearrange("(ko p) t -> p ko t", p=128)   # [128, 8, 1536]
    xlo_r = xlo.rearrange("(ko p) t -> p ko t", p=128)
    w_r = {n: t.rearrange("(ko p) d -> p ko d", p=128)
           for n, t in (("wqh", wqh), ("wql", wql), ("wkh", wkh),
                        ("wkl", wkl), ("wvh", wvh), ("wvl", wvl),
                        ("wo", woT))}
    out_r = out.rearrange("(to p) d -> p to d", p=128)   # [128, 8, 1024]

    # per-exp PE budget: exp [128,1024] = 1038 ns = 2491 PE cycles;
    # scores per kt = 1024 cy, one pipelined AV pass = 1040 cy.
    EXP_CY = 2491
    SCORE_CY = 1024
    AV_CY = 1040
    WSCALE_INV = 1.0 / 64.0

    with tile.TileContext(nc) as tc:
        with (
            tc.tile_pool(name="const", bufs=1) as constp,
            tc.tile_pool(name="xw", bufs=1) as xwp,
            tc.tile_pool(name="w8", bufs=1) as w8p,
            tc.tile_pool(name="wo", bufs=1) as wop,
            tc.tile_pool(name="acts", bufs=1) as actp,
            tc.tile_pool(name="ptiles", bufs=12) as pp,
            tc.tile_pool(name="ctxn", bufs=4) as ctxp,
            tc.tile_pool(name="recs", bufs=6) as recp,
            tc.tile_pool(name="outs", bufs=2) as op,
            tc.tile_pool(name="psS", bufs=2, space="PSUM") as psS,
            tc.tile_pool(name="psV", bufs=3, space="PSUM") as psV,
            tc.tile_pool(name="psA", bufs=1, space="PSUM") as psA,
        ):
            # ---- input DMA: own-token data first (chunk 1 runs first
            # and touches no halo); few, large transfers (HWDGE passes
            # serialize at ~630 ns each) ----
            bq_sb = constp.tile([128, 8], f32)
            bk_sb = constp.tile([128, 8], f32)
            vb_sb = constp.tile([128, 1], f32)
            bo_sb = constp.tile([128, D], bf16)
            nc.scalar.dma_start(bq_sb[:], bqr[:])
            nc.scalar.dma_start(bk_sb[:], bkr[:])
            nc.scalar.dma_start(vb_sb[:], vbias[:])
            nc.scalar.dma_start(bo_sb[:], borep[:])

            xh_sb = xwp.tile([128, 8, NKV], fp8, tag="xh")
            xl_sb = xwp.tile([128, 8, NKV], fp8, tag="xl")
            w_sb = {n: w8p.tile([128, 8, D], fp8, tag=n, name=n)
                    for n in ("wqh", "wql", "wkh", "wkl", "wvh", "wvl")}
            def wload(n, c0, c1):
                nc.sync.dma_start(w_sb[n][:, :, c0:c1], w_r[n][:, :, c0:c1])

            wload("wqh", 0, 128)
            nc.sync.dma_start(xh_sb[:, 0:2, 512:1536], xhi_r[:, 0:2, 512:1536])
            nc.sync.dma_start(xh_sb[:, 2:4, 512:1536], xhi_r[:, 2:4, 512:1536])
            nc.sync.dma_start(xh_sb[:, 4:6, 512:1536], xhi_r[:, 4:6, 512:1536])
            nc.sync.dma_start(xh_sb[:, 6:8, 512:1536], xhi_r[:, 6:8, 512:1536])
            nc.sync.dma_start(xl_sb[:, 0:4, 512:1536], xlo_r[:, 0:4, 512:1536])
            nc.sync.dma_start(xl_sb[:, 4:8, 512:1536], xlo_r[:, 4:8, 512:1536])
            wload("wql", 0, 128)
            wload("wkh", 0, 128)
            wload("wkl", 0, 128)
            wload("wqh", 128, 512)
            wload("wql", 128, 512)
            wload("wkh", 128, 1024)
            wload("wkl", 128, 1024)
            wload("wvh", 0, 1024)
            wload("wvl", 0, 1024)
            wload("wqh", 512, 1024)
            wload("wql", 512, 1024)
            nc.scalar.dma_start(xh_sb[:, :, 0:512], xhi_r[:, :, 0:512])
            nc.scalar.dma_start(xl_sb[:, :, 0:512], xlo_r[:, :, 0:512])

            # ---- persistent activations ----
            q_sb = actp.tile([128, 8, BLK], bf16, tag="q")    # q^T [d, tok]
            k_sb = actp.tile([128, 8, NKV], bf16, tag="k")    # k^T [d, tok]
            # v natural [tok, h, dh+1]; col 64 per head = ones column that
            # accumulates the softmax denominator during AV.
            v_sb = actp.tile([128, 12, H * (DH + 1)], bf16, tag="v")
            v_v = v_sb[:].rearrange("p t (h e) -> p t h e", e=DH + 1)
            nc.vector.memset(v_v[:, :, :, DH], 1.0)
            # ctx^T [d, tok] for the out-projection (filled by dma xbar)
            ctxT_sb = actp.tile([128, 8, BLK], bf16, tag="ctxT")

            # ---- projections (fp8 DoubleRow hi/lo) ----
            # psA has a single bank; always alternate with a psV bank so a
            # unit's matmuls overlap the previous unit's DVE drain (a 1-ring
            # serializes on the write-after-read and costs ~950 ns/unit).
            ps_rot = [psA, psV]
            rot_i = [0]

            def dr_group(ps, lhs_hi, lhs_lo, rhs_hi, rhs_lo, lsl, rsl):
                """12 DR matmuls: hi*hi (4 ko-pair steps) + hi*lo + lo*hi."""
                for term, (lh, rh) in enumerate(
                        ((lhs_hi, rhs_hi), (lhs_hi, rhs_lo), (lhs_lo, rhs_hi))):
                    for s in range(4):
                        ksl = slice(2 * s, 2 * s + 2)
                        nc.tensor.matmul(
                            ps[:], lh[:, ksl, lsl], rh[:, ksl, rsl],
                            start=(term == 0 and s == 0),
                            stop=(term == 2 and s == 3), perf_mode=DR)

            def proj_ps(borrow=True):
                pool = ps_rot[rot_i[0] % 2]
                rot_i[0] += 1
                tag = "ps" if pool is psA else "av"
                return pool.tile([128, 512], f32, name="bps", tag=tag)

            def proj_qk_m(wh, wl, dst, bias, xn, dn, m, borrow=False):
                """One [128 dout x 512 tok] tile of a q^T/k^T projection."""
                ps = proj_ps(borrow)
                dr_group(ps, w_sb[wh], w_sb[wl], xh_sb[:], xl_sb[:],
                         slice(m * 128, (m + 1) * 128),
                         slice(xn * 512, (xn + 1) * 512))
                nc.vector.tensor_scalar(
                    dst[:, m, dn * 512:dn * 512 + 512], ps[:], WSCALE_INV,
                    bias[:, m:m + 1], mybir.AluOpType.mult,
                    mybir.AluOpType.add)

            def proj_v_t(t, n, borrow=False):
                """One [128 tok x 512 feature] tile of the v projection."""
                ps = proj_ps(borrow)
                dr_group(ps, xh_sb[:], xl_sb[:], w_sb["wvh"], w_sb["wvl"],
                         slice(t * 128, (t + 1) * 128),
                         slice(n * 512, (n + 1) * 512))
                nc.vector.tensor_scalar_mul(
                    v_v[:, t, n * 8:(n + 1) * 8, :DH],
                    ps[:].rearrange("p (h e) -> p h e", e=DH), WSCALE_INV)

            # ---- filler: PE work interleaved into the attention stream ----
            filler = []
            state = {"deficit": 0, "idx": 0}

            def add_filler(cycles, fn):
                filler.append((cycles, fn))

            def drain(cycles):
                state["deficit"] += cycles
                while (state["idx"] < len(filler)
                       and state["deficit"] >= filler[state["idx"]][0]):
                    cyc, fn = filler[state["idx"]]
                    state["idx"] += 1
                    state["deficit"] -= cyc
                    fn()

            def drain_to(idx):
                while state["idx"] < min(idx, len(filler)):
                    state["deficit"] = 0
                    cyc, fn = filler[state["idx"]]
                    state["idx"] += 1
                    fn()

            # ---- attention ----
            ctx_slabs = {}

            def transpose(c, qt):
                nc.sync.dma_start_transpose(
                    ctxT_sb[:, :, (c * 4 + qt) * 128:(c * 4 + qt + 1) * 128],
                    ctx_slabs[(c, qt)][:])

            def av_pass(c, u, plist, g, qta, qtb):
                """One AV pass: heads 2u+g, query tiles (qta, qtb)."""
                h = 2 * u + g
                tiles = []
                for qt in (qta, qtb):
                    tiles.append(psV.tile([128, 512], f32, name="av", tag="av"))
                for kt in range(KT):
                    vsl = v_v[:, c * 4 + kt, h, :]
                    for qt, av in zip((qta, qtb), tiles):
                        nc.tensor.matmul(
                            av[:, 0:65],
                            plist[kt][:, g * 512 + qt * 128:
                                      g * 512 + (qt + 1) * 128],
                            vsl, start=(kt == 0), stop=(kt == KT - 1))
                for qt, av in zip((qta, qtb), tiles):
                    rec = recp.tile([128, 1], f32, tag="rec", name="rec")
                    nc.vector.reciprocal(rec[:], av[:, 64:65])
                    nc.vector.tensor_scalar_mul(
                        ctx_slabs[(c, qt)][:, h * 64:(h + 1) * 64],
                        av[:, 0:64], rec[:, 0:1])

            PASSES = ((0, 0, 1), (1, 0, 1), (0, 2, 3), (1, 2, 3))
            pending = [None]      # (c, u, plist) awaiting AV
            pair_no = [0]

            def flush_pending(kt=None):
                """Emit AV pass kt (or all remaining) of the pending pair;
                after the chunk's last pair, transposes chase the passes."""
                if pending[0] is None:
                    return
                c, u, plist, done = pending[0]
                rng = range(4) if kt is None else [kt]
                for i in rng:
                    if i < done:
                        continue
                    av_pass(c, u, plist, *PASSES[i])
                    pending[0] = (c, u, plist, i + 1)
                    if u == 7 and i == 1:
                        transpose(c, 0)
                        transpose(c, 1)
                    if u == 7 and i == 3:
                        transpose(c, 2)
                        transpose(c, 3)
                if pending[0][3] >= 4:
                    pending[0] = None

            def emit_pair(c, u, force_idx=None):
                if force_idx is not None:
                    drain_to(force_idx)
                plist = []
                for kt in range(KT):
                    ksl = slice(c * 512 + kt * 128, c * 512 + (kt + 1) * 128)
                    qsl = slice(c * 512, (c + 1) * 512)
                    sps = psS.tile([128, 1024], f32, name="sps")
                    nc.tensor.matmul(sps[:, 0:512], k_sb[0:64, u, ksl],
                                     q_sb[0:64, u, qsl], start=True, stop=True)
                    nc.tensor.matmul(sps[:, 512:1024], k_sb[64:128, u, ksl],
                                     q_sb[64:128, u, qsl], start=True,
                                     stop=True)
                    p_t = pp.tile([128, 1024], bf16, tag="p", name="p")
                    if c == 0 and kt < 4:
                        nc.scalar.activation(p_t[:], sps[:], AF.Exp,
                                             bias=vb_sb[:, 0:1], scale=0.125)
                    else:
                        nc.scalar.activation(p_t[:], sps[:], AF.Exp,
                                             scale=0.125)
                    plist.append(p_t)
                    # early pairs are DMA-gated; mid pairs drain hard (ACT
                    # has slack); late pairs keep filler for the ACT-paced
                    # endgame.
                    pn = pair_no[0]
                    budget = 1300 if pn < 1 else 1550
                    if kt < 4 and pending[0] is not None:
                        pc, pu = pending[0][0], pending[0][1]
                        if kt == 0 and (pc, pu) in av_force:
                            drain_to(av_force[(pc, pu)])
                        flush_pending(kt)
                        budget -= AV_CY
                    drain(max(budget, 0))
                flush_pending()   # no-op unless fewer than 4 slots were free
                pending[0] = (c, u, plist, 0)
                pair_no[0] += 1

            def out_proj(to, n, borrow=False):
                ps = proj_ps(borrow)
                for ko in range(8):
                    nc.tensor.matmul(
                        ps[:], ctxT_sb[:, ko, to * 128:(to + 1) * 128],
                        wo_sb[:, ko, n * 512:(n + 1) * 512],
                        start=(ko == 0), stop=(ko == 7))
                o_t = op.tile([128, 512], f32, tag="o", name="o")
                nc.vector.tensor_add(o_t[:], ps[:],
                                     bo_sb[:, n * 512:(n + 1) * 512])
                nc.sync.dma_start(out_r[:, to, n * 512:(n + 1) * 512], o_t[:])

            # ---- phase schedule ----
            # Chunk 1 first: it touches no halo data, so the DMA stream
            # delivers own-token x/w first and the halo trails. All
            # projections beyond the first three m-tiles ride as filler
            # inside the attention stream (the exp stream is the pacer).
            wo_sb = wop.tile([128, 8, D], bf16)
            nc.sync.dma_start(wo_sb[:], w_r["wo"][:])

            # pre-attention: just what (1,0)'s first score tiles need;
            # K-s2-m0 (k-tiles 4..7) rides at the head of the filler.
            proj_qk_m("wqh", "wql", q_sb, bq_sb, 2, 1, 0, borrow=True)
            proj_qk_m("wkh", "wkl", k_sb, bk_sb, 1, 1, 0, borrow=True)

            score_force = {}
            av_force = {}
            add_filler(3072, lambda: proj_qk_m(
                "wkh", "wkl", k_sb, bk_sb, 2, 2, 0))

            def add_qk1(u):
                add_filler(3072, lambda m=u: proj_qk_m(
                    "wqh", "wql", q_sb, bq_sb, 2, 1, m))
                add_filler(3072, lambda m=u: proj_qk_m(
                    "wkh", "wkl", k_sb, bk_sb, 1, 1, m))
                add_filler(3072, lambda m=u: proj_qk_m(
                    "wkh", "wkl", k_sb, bk_sb, 2, 2, m))
                score_force[(1, u)] = len(filler)

            def add_qk0(u):
                add_filler(3072, lambda m=u: proj_qk_m(
                    "wqh", "wql", q_sb, bq_sb, 1, 0, m))
                add_filler(3072, lambda m=u: proj_qk_m(
                    "wkh", "wkl", k_sb, bk_sb, 0, 0, m))
                score_force[(0, u)] = len(filler)

            add_qk1(1)
            for t in range(4, 12):
                add_filler(3072, lambda t=t: proj_v_t(t, 0))
            av_force[(1, 0)] = len(filler)
            add_qk1(2)
            add_qk1(3)
            add_qk1(4)
            for t in range(4, 12):
                add_filler(3072, lambda t=t: proj_v_t(t, 1))
            av_force[(1, 4)] = len(filler)
            for u in (5, 6, 7):
                add_qk1(u)
            for u in range(8):
                add_qk0(u)
            for t in range(4):
                add_filler(3072, lambda t=t: proj_v_t(t, 0))
            av_force[(0, 0)] = len(filler)
            for t in range(4):
                add_filler(3072, lambda t=t: proj_v_t(t, 1))
            av_force[(0, 4)] = len(filler)
            for to in range(4, 8):
                for n in range(2):
                    add_filler(4096, lambda to=to, n=n: out_proj(to, n))

            for qt in range(4):
                ctx_slabs[(1, qt)] = ctxp.tile([128, BLK], bf16, tag="slab",
                                               name="slab")
            for u in range(8):
                emit_pair(1, u, force_idx=score_force.get((1, u)))
            for qt in range(4):
                ctx_slabs[(0, qt)] = ctxp.tile([128, BLK], bf16, tag="slab",
                                               name="slab")
            for u in range(8):
                emit_pair(0, u, force_idx=score_force.get((0, u)))

            flush_pending()
            drain_to(len(filler))
            for to in range(4):
                for n in range(2):
                    out_proj(to, n, borrow=True)

    nc.compile()
    return nc


def _host_prep(x, Wq, bq, Wk, bk, Wv, bv, Wo, bo):
    import ml_dtypes

    e4 = ml_dtypes.float8_e4m3
    bf = ml_dtypes.bfloat16

    def split8(a):
        a = np.ascontiguousarray(a, dtype=np.float32)
        hi = a.astype(e4)
        lo = (a - hi.astype(np.float32)).astype(e4)
        return hi, lo

    x = np.ascontiguousarray(np.asarray(x, dtype=np.float32))
    Wq = np.asarray(Wq, np.float32)
    Wk = np.asarray(Wk, np.float32)
    Wv = np.asarray(Wv, np.float32)
    Wo = np.asarray(Wo, np.float32)
    bv = np.asarray(bv, np.float32)
    bo = np.asarray(bo, np.float32)

    wqh, wql = split8(Wq.T * 64.0)
    wkh, wkl = split8(Wk.T * 64.0)
    wvh, wvl = split8(Wv.T * 64.0)
    bo_eff = bo + Wo @ bv          # v-bias folded through the attention avg
    mats = {
        "wqh": wqh, "wql": wql, "wkh": wkh, "wkl": wkl,
        "wvh": wvh, "wvl": wvl,
        "woT": np.ascontiguousarray(Wo.T.astype(bf)),
        "bqr": np.ascontiguousarray(
            np.asarray(bq, np.float32).reshape(8, 128).T),
        "bkr": np.ascontiguousarray(
            np.asarray(bk, np.float32).reshape(8, 128).T),
        "borep": np.ascontiguousarray(
            np.tile(bo_eff[None, :], (128, 1)).astype(bf)),
    }

    in_maps = []
    for core in range(NCORES):
        b, j = core // 4, core % 4
        start = j * BLK
        xkv = np.zeros((NKV, D), np.float32)
        lo = start - W
        if lo < 0:
            xkv[W:] = x[b, start:start + BLK]
        else:
            xkv[:] = x[b, lo:start + BLK]
        xh, xl = split8(xkv.T)
        vb = np.zeros((128, 1), np.float32)
        if j == 0:
            vb[:] = -1e9           # chunk-0 halo k-tiles masked in the exp
        im = dict(mats)
        im["xhi"] = xh
        im["xlo"] = xl
        im["vbias"] = vb
        in_maps.append(im)
    return in_maps


def kernel(x, Wq, bq, Wk, bk, Wv, bv, Wo, bo):
    from concourse.bass_utils import run_bass_kernel_spmd

    if "nc" not in _CACHE:
        _CACHE["nc"] = _build()
    nc = _CACHE["nc"]

    in_maps = _host_prep(x, Wq, bq, Wk, bk, Wv, bv, Wo, bo)
    res = run_bass_kernel_spmd(nc, in_maps, list(range(NCORES)))

    out = np.empty((B, L, D), np.float32)
    for core in range(NCORES):
        b, j = core // 4, core % 4
        out[b, j * BLK:(j + 1) * BLK] = res.results[core]["out"]
    return out
